# revision 9
# baseline (speedup 1.0000x reference)
"""DiceCE loss kernel for Trainium2 (8 NeuronCores, SPMD spatial sharding).

Computes (faithfully to the reference's cross-batch one-hot CE):
  logp_sum[n,s] = sum_b log(pred[b,n,s] + EPS)
  ce = -mean_{b,s}(logp_sum[t[b,s], s]) / B
  dice = mean_{b,n}(1 - (2*inter + SM) / (ground_o + pred_o + SM))
  loss = ce + dice

Strategy: shard the flattened spatial grid (H*W*D = 2^21) across the 8 cores;
each core holds BOTH batches for its spatial chunk, so the cross-batch CE
coupling is purely core-local and no collective is needed. Each core emits a
[128, 64] f32 partial-stats tile (ground_o / inter / ce / pred_o per (b,n)),
reduced and combined into the scalar loss on the host.

Inputs are converted to bf16 on the host (halves HBM traffic; all on-device
reductions accumulate in f32 via accum_out).
"""

import sys

sys.path.insert(0, "/opt/trn_rl_repo")

import functools

import numpy as np
import ml_dtypes

import concourse.bass as bass
import concourse.bacc as bacc
import concourse.tile as tile
from concourse import mybir
from concourse import bass_utils

B, N = 2, 8
H = W = D = 128
HWD = H * W * D            # 2097152
NCORES = 8
S = HWD // NCORES          # 262144 spatial positions per core
P = 128                    # SBUF partitions
F = S // P                 # 2048 free elements per tile
EPS = 1e-10
SMOOTH = 1e-5

BF16 = mybir.dt.bfloat16
F32 = mybir.dt.float32
ALU = mybir.AluOpType
ACTF = mybir.ActivationFunctionType

# stats tile column layout: [0:16] ground_o, [16:32] inter, [32:48] ce, [48:64] pred_o
# index within a group: idx = b*N + n


def _build_nc() -> bass.Bass:
    # Bacc (not raw Bass): its compile() runs generate_event_semaphores, which
    # splits multi-wait sync conditions to satisfy the 1-wait-per-instruction
    # TRN2 codegen constraint.
    nc = bacc.Bacc(
        "TRN2", target_bir_lowering=False, debug=False, enable_asserts=False
    )
    pred = nc.dram_tensor("pred", [B * N, P, F], BF16, kind="ExternalInput").ap()
    targ = nc.dram_tensor("targ", [B, P, F], BF16, kind="ExternalInput").ap()
    stats = nc.dram_tensor("stats", [P, 64], F32, kind="ExternalOutput").ap()

    with tile.TileContext(nc) as tc:
        with (
            tc.tile_pool(name="tpool", bufs=1) as tpool,
            tc.tile_pool(name="ppool", bufs=4) as ppool,
            tc.tile_pool(name="lgpool", bufs=3) as lgpool,
            tc.tile_pool(name="mpool", bufs=3) as mpool,
            tc.tile_pool(name="cpool", bufs=2) as cpool,
            tc.tile_pool(name="spool", bufs=4) as spool,
            tc.tile_pool(name="stpool", bufs=1) as stpool,
        ):
            st = stpool.tile([P, 64], F32, name="st")
            nc.vector.memset(st, 0.0)

            eps_t = stpool.tile([P, 1], F32, name="eps_t")
            nc.vector.memset(eps_t, EPS)

            t_tiles = []
            for b in range(B):
                tt = tpool.tile([P, F], BF16, name=f"t{b}")
                nc.sync.dma_start(out=tt, in_=targ[b])
                t_tiles.append(tt)

            for n in range(N):
                p_t, lg_t, m_t = [], [], []
                for b in range(B):
                    idx = b * N + n
                    pt = ppool.tile([P, F], BF16, name="pt", tag="pt")
                    nc.sync.dma_start(out=pt, in_=pred[idx])
                    # lg = log(pred + EPS)
                    lg = lgpool.tile([P, F], BF16, name="lg", tag="lg")
                    nc.scalar.activation(lg, pt, ACTF.Ln, bias=eps_t)
                    # mask = (t == n), ground_o = sum(mask)
                    m = mpool.tile([P, F], BF16, name="m", tag="m")
                    nc.vector.tensor_scalar(
                        out=m,
                        in0=t_tiles[b],
                        scalar1=float(n),
                        scalar2=None,
                        op0=ALU.is_equal,
                        op1=ALU.add,
                        accum_out=st[:, idx : idx + 1],
                    )
                    # pred_o = sum(pred)
                    sc1 = spool.tile([P, F], BF16, name="sc1", tag="sc")
                    nc.vector.tensor_scalar(
                        out=sc1,
                        in0=pt,
                        scalar1=1.0,
                        scalar2=None,
                        op0=ALU.mult,
                        op1=ALU.add,
                        accum_out=st[:, 48 + idx : 49 + idx],
                    )
                    p_t.append(pt)
                    lg_t.append(lg)
                    m_t.append(m)

                # cnt = m0 + m1  (values 0/1/2, exact in bf16)
                cnt = cpool.tile([P, F], BF16, name="cnt", tag="cnt")
                nc.vector.tensor_tensor(out=cnt, in0=m_t[0], in1=m_t[1], op=ALU.add)

                for b in range(B):
                    idx = b * N + n
                    # inter[b,n] = sum(mask * pred)  via scalar_tensor_tensor:
                    # out = (m * 1.0) * p, accum = sum(out)
                    sc2 = spool.tile([P, F], BF16, name="sc2", tag="sc")
                    nc.vector.scalar_tensor_tensor(
                        out=sc2,
                        in0=m_t[b],
                        scalar=1.0,
                        in1=p_t[b],
                        op0=ALU.mult,
                        op1=ALU.mult,
                        accum_out=st[:, 16 + idx : 17 + idx],
                    )
                    # ce[b,n] = sum(cnt * lg_b)
                    sc3 = spool.tile([P, F], BF16, name="sc3", tag="sc")
                    nc.vector.scalar_tensor_tensor(
                        out=sc3,
                        in0=cnt,
                        scalar=1.0,
                        in1=lg_t[b],
                        op0=ALU.mult,
                        op1=ALU.mult,
                        accum_out=st[:, 32 + idx : 33 + idx],
                    )

            nc.sync.dma_start(out=stats, in_=st)
    nc.compile()
    return nc


@functools.lru_cache(maxsize=1)
def _get_nc() -> bass.Bass:
    return _build_nc()


def _make_in_maps(pred: np.ndarray, target: np.ndarray) -> list[dict[str, np.ndarray]]:
    pred_bf = np.asarray(pred).reshape(B, N, HWD).astype(ml_dtypes.bfloat16)
    targ_bf = np.asarray(target).reshape(B, HWD).astype(ml_dtypes.bfloat16)
    in_maps = []
    for c in range(NCORES):
        sl = slice(c * S, (c + 1) * S)
        pm = np.ascontiguousarray(pred_bf[:, :, sl]).reshape(B * N, P, F)
        tm = np.ascontiguousarray(targ_bf[:, sl]).reshape(B, P, F)
        in_maps.append({"pred": pm, "targ": tm})
    return in_maps


def _combine(stats_per_core: list[np.ndarray]) -> np.float32:
    gnd = np.zeros((B, N), np.float64)
    inter = np.zeros((B, N), np.float64)
    predo = np.zeros((B, N), np.float64)
    ce_total = 0.0
    for stc in stats_per_core:
        s = stc.astype(np.float64).sum(axis=0)  # [64]
        gnd += s[0:16].reshape(B, N)
        inter += s[16:32].reshape(B, N)
        ce_total += s[32:48].sum()
        predo += s[48:64].reshape(B, N)
    celoss = -ce_total / (B * HWD) / B
    dice = np.mean(1.0 - (2.0 * inter + SMOOTH) / (gnd + predo + SMOOTH))
    return np.float32(celoss + dice)


def kernel(pred: np.ndarray, target: np.ndarray) -> np.ndarray:
    nc = _get_nc()
    in_maps = _make_in_maps(pred, target)
    res = bass_utils.run_bass_kernel_spmd(nc, in_maps, core_ids=list(range(NCORES)))
    return _combine([r["stats"] for r in res.results])


# Used by test.py for profiling access to the raw results object.
def run_raw(pred: np.ndarray, target: np.ndarray, **kwargs) -> bass_utils.BassKernelResults:
    nc = _get_nc()
    in_maps = _make_in_maps(pred, target)
    return bass_utils.run_bass_kernel_spmd(
        nc, in_maps, core_ids=list(range(NCORES)), **kwargs
    )
